# revision 9
# baseline (speedup 1.0000x reference)
"""KeyedGRU Trainium2 Bass kernel.

Strategy: data-parallel over batch B=64 across 8 cores (B=8 each), weights
replicated. Per core:
  Phase 0: 16-step key-gate GRU scan (KB=4) -> per-step gates g[16, H].
  Phase 1: 2048-step main GRU. The input-side matmul gi = x @ W_ih.T + bias
  is precomputed in 32-step chunks on the tensor engine (independent of h)
  and interleaved into the per-step idle windows; the sequential per-step
  work is gh = h @ W_hh.T (12 small matmuls, H-on-partitions layout),
  one sigmoid pass (r,i), the n-gate tanh chain on DVE/ACT, and the lerp.
Layouts keep H on SBUF partitions so elementwise ops run on [128, ~16-32]
tiles; output is staged [128, 2, B, 128] and DMA'd as [ht, p, b, t]; the
host reassembles [T, B, H].
"""
import numpy as np
import concourse.bass as bass
import concourse.tile as tile
from concourse import mybir
from concourse.bass_utils import run_bass_kernel_spmd

f32 = mybir.dt.float32
AF = mybir.ActivationFunctionType
ALU = mybir.AluOpType

B, T_FULL, I, H = 64, 2048, 256, 256
KB, KL = 4, 16
NCORE = 8
BC = B // NCORE          # batch per core
M3 = 3 * H               # 768 gate outputs
CH = 32                  # gi chunk (steps)
OCH = 128                # output chunk (steps)


def _fix_waits(nc, limit=1):
    """walrus TPB_CTRL encodes only one sync-wait; split extras onto nops."""
    for func in nc.m.functions:
        for bb in func.blocks:
            out = []
            for ins in bb.instructions:
                si = ins.sync_info
                if si and len(si.on_wait) > limit:
                    waits = list(si.on_wait)
                    for j, w in enumerate(waits[:-limit]):
                        nop = mybir.InstNoOp(name=f"{ins.name}-wfix{j}", ins=[], outs=[])
                        nop.engine = ins.engine
                        nop.sync_info = mybir.SyncInfo(on_wait=[w], on_update=[])
                        out.append(nop)
                    ins.sync_info = mybir.SyncInfo(
                        on_wait=list(waits[-limit:]), on_update=list(si.on_update)
                    )
                out.append(ins)
            bb.instructions = out


def _build(T):
    NCH = T // CH
    nc = bass.Bass("TRN2", num_devices=NCORE)
    x_in = nc.declare_dram_parameter("x", [2, 128, T, BC], f32, isOutput=False)
    wih_d = nc.declare_dram_parameter("wih", [2, 128, M3], f32, isOutput=False)
    whh_d = nc.declare_dram_parameter("whh", [2, 128, M3], f32, isOutput=False)
    brow_d = nc.declare_dram_parameter("brow", [1, M3], f32, isOutput=False)
    bhn_d = nc.declare_dram_parameter("bhn", [2, 128, BC], f32, isOutput=False)
    wmk_d = nc.declare_dram_parameter("wmk", [2, 128, KL * KB], f32, isOutput=False)
    out_d = nc.declare_dram_parameter("out", [2, 128, BC, T], f32, isOutput=True)

    with tile.TileContext(nc) as tc:
        with (
            tc.tile_pool(name="const", bufs=1) as const,
            tc.tile_pool(name="xin", bufs=3) as xin,
            tc.tile_pool(name="gips", bufs=2, space="PSUM") as gips,
            tc.tile_pool(name="ghps", bufs=2, space="PSUM") as ghps,
            tc.tile_pool(name="gisb", bufs=2) as gisb,
            tc.tile_pool(name="outb", bufs=2) as outb,
            tc.tile_pool(name="tmp", bufs=3) as tmp,
        ):
            # ---- constants ----
            wih_sb = const.tile([128, 2, M3], f32)
            whh_sb = const.tile([128, 2, M3], f32)
            for k in range(2):
                nc.sync.dma_start(out=wih_sb[:, k, :], in_=wih_d[k])
                nc.sync.dma_start(out=whh_sb[:, k, :], in_=whh_d[k])
            brow_sb = const.tile([1, M3], f32)
            nc.sync.dma_start(out=brow_sb, in_=brow_d[:, :])
            bhn_sb = const.tile([128, 2, BC], f32)
            for k in range(2):
                nc.sync.dma_start(out=bhn_sb[:, k, :], in_=bhn_d[k])
            kx_sb = const.tile([128, 2, KL * KB], f32)
            for k in range(2):
                nc.sync.dma_start(out=kx_sb[:, k, :], in_=wmk_d[k])
            ones_sb = const.tile([1, CH * BC], f32)
            nc.vector.memset(ones_sb, 1.0)
            rbuf = const.tile([128, 2, KL, KB], f32)   # reset gates, key scan
            gr_sb = const.tile([128, 2, KL], f32)
            g_sb = const.tile([128, 2, KL], f32)
            h0 = const.tile([128, 2, BC], f32)
            nc.vector.memset(h0, 0.0)
            kgi_sb = const.tile([128, 6, KL * KB], f32)

            def mm(out_ap, lhsT, rhs, start, stop):
                nc.tensor.matmul(out_ap, lhsT, rhs, start=start, stop=stop)

            # ---- phase 0: key-gate scan (KB=4, KL=16) ----
            kgi_ps = gips.tile([128, 6, KL * KB], f32, tag="gi")
            for m in range(6):
                sl = slice(m * 128, (m + 1) * 128)
                mm(kgi_ps[:, m, :], wih_sb[:, 0, sl], kx_sb[:, 0, :], True, False)
                mm(kgi_ps[:, m, :], wih_sb[:, 1, sl], kx_sb[:, 1, :], False, False)
                mm(kgi_ps[:, m, :], brow_sb[:, sl], ones_sb[:, : KL * KB], False, True)
            nc.vector.tensor_copy(kgi_sb, kgi_ps)

            kh = tmp.tile([128, 2, KB], f32, tag="kh")
            nc.vector.memset(kh, 0.0)
            for t in range(KL):
                ksl = slice(t * KB, (t + 1) * KB)
                kgh = ghps.tile([128, 6, KB], f32, tag="gh")
                for m in range(6):
                    sl = slice(m * 128, (m + 1) * 128)
                    mm(kgh[:, m, :], whh_sb[:, 0, sl], kh[:, 0, :], True, False)
                    mm(kgh[:, m, :], whh_sb[:, 1, sl], kh[:, 1, :], False, True)
                sri = tmp.tile([128, 4, KB], f32, tag="sri")
                nc.vector.tensor_add(sri, kgh[:, 0:4, :], kgi_sb[:, 0:4, ksl])
                sig = tmp.tile([128, 4, KB], f32, tag="sig")
                nc.scalar.activation(sig, sri, AF.Sigmoid)
                nc.vector.tensor_copy(rbuf[:, :, t, :], sig[:, 0:2, :])
                t1 = tmp.tile([128, 2, KB], f32, tag="t1")
                nc.vector.tensor_add(t1, kgh[:, 4:6, :], bhn_sb[:, :, 0:KB])
                t2 = tmp.tile([128, 2, KB], f32, tag="t2")
                nc.vector.tensor_mul(t2, t1, sig[:, 0:2, :])
                t3 = tmp.tile([128, 2, KB], f32, tag="t3")
                nc.vector.tensor_add(t3, t2, kgi_sb[:, 4:6, ksl])
                nn = tmp.tile([128, 2, KB], f32, tag="nn")
                nc.scalar.activation(nn, t3, AF.Tanh)
                dd = tmp.tile([128, 2, KB], f32, tag="dd")
                nc.vector.tensor_sub(dd, kh, nn)
                ee = tmp.tile([128, 2, KB], f32, tag="ee")
                nc.vector.tensor_mul(ee, dd, sig[:, 2:4, :])
                kh2 = tmp.tile([128, 2, KB], f32, tag="kh")
                nc.vector.tensor_add(kh2, ee, nn)
                kh = kh2
            nc.vector.tensor_reduce(gr_sb, rbuf, axis=mybir.AxisListType.X, op=ALU.add)
            nc.vector.tensor_scalar_mul(g_sb, gr_sb, 1.0 / KB)

            # ---- phase 1: main recurrence ----
            x_tiles, gi_ps_t, gi_sb_t = {}, {}, {}
            pending = []  # deferred GI emission ops: ("mm", c, m, kk) | ("cp", c)

            def emit_x(c):
                xt = xin.tile([128, 2, CH, BC], f32, tag="x", name=f"x{c}")
                sl = slice(c * CH, (c + 1) * CH)
                for k in range(2):
                    nc.sync.dma_start(out=xt[:, k, :, :], in_=x_in[k, :, sl, :])
                x_tiles[c] = xt

            def queue_gi(c):
                gi_ps_t[c] = gips.tile([128, 6, CH * BC], f32, tag="gi", name=f"gi_ps{c}")
                gi_sb_t[c] = gisb.tile([128, 6, CH * BC], f32, tag="gis", name=f"gi_sb{c}")
                for m in range(6):
                    for kk in range(3):
                        pending.append(("mm", c, m, kk))
                pending.append(("cp", c))

            def emit_gi_op(op):
                _, c, m, kk = op if op[0] == "mm" else (None, op[1], None, None)
                if op[0] == "mm":
                    sl = slice(m * 128, (m + 1) * 128)
                    tgt = gi_ps_t[c][:, m, :]
                    if kk < 2:
                        mm(tgt, wih_sb[:, kk, sl], x_tiles[c][:, kk, :, :], kk == 0, False)
                    else:
                        mm(tgt, brow_sb[:, sl], ones_sb, False, True)
                else:
                    nc.vector.tensor_copy(gi_sb_t[c], gi_ps_t[c])

            # chunk 0 fully up-front; chunk 1 queued so it fills phase-0/early gaps
            emit_x(0)
            queue_gi(0)
            while pending:
                emit_gi_op(pending.pop(0))
            if NCH > 1:
                emit_x(1)
                queue_gi(1)

            hcur = lambda k: h0[:, k, :]      # per-Htile matmul rhs view
            hfull = h0[:, :, :]               # full [128, 2, BC] view for DVE
            ob = None
            for t in range(T):
                c, o = divmod(t, CH)
                ot = t % OCH
                osl = slice(o * BC, (o + 1) * BC)
                if t % OCH == 0:
                    ob = outb.tile([128, 2, BC, OCH], f32, tag="ob")
                if t % CH == 0 and c + 2 < NCH:
                    emit_x(c + 2)
                    queue_gi(c + 2)
                gh = ghps.tile([128, 6, BC], f32, tag="gh")
                for m in range(6):
                    sl = slice(m * 128, (m + 1) * 128)
                    mm(gh[:, m, :], whh_sb[:, 0, sl], hcur(0), True, False)
                    mm(gh[:, m, :], whh_sb[:, 1, sl], hcur(1), False, True)
                # fill PE idle windows with next chunk's gi work
                for _ in range(2):
                    if pending:
                        emit_gi_op(pending.pop(0))
                gsb = gi_sb_t[c]
                sri = tmp.tile([128, 4, BC], f32, tag="sri")
                nc.vector.tensor_add(sri, gh[:, 0:4, :], gsb[:, 0:4, osl])
                sig = tmp.tile([128, 4, BC], f32, tag="sig")
                nc.scalar.activation(sig, sri, AF.Sigmoid)
                t1 = tmp.tile([128, 2, BC], f32, tag="t1")
                nc.vector.tensor_add(t1, gh[:, 4:6, :], bhn_sb)
                t2 = tmp.tile([128, 2, BC], f32, tag="t2")
                nc.vector.tensor_mul(t2, t1, sig[:, 0:2, :])
                t3 = tmp.tile([128, 2, BC], f32, tag="t3")
                nc.vector.tensor_add(t3, t2, gsb[:, 4:6, osl])
                nn = tmp.tile([128, 2, BC], f32, tag="nn")
                nc.scalar.activation(nn, t3, AF.Tanh)
                dd = tmp.tile([128, 2, BC], f32, tag="dd")
                nc.vector.tensor_sub(dd, hfull, nn)
                ee = tmp.tile([128, 2, BC], f32, tag="ee")
                nc.vector.tensor_mul(ee, dd, sig[:, 2:4, :])
                nc.vector.tensor_add(ob[:, :, :, ot], ee, nn)
                if t < KL:
                    hg = tmp.tile([128, 2, BC], f32, tag="hg")
                    for k in range(2):
                        nc.vector.tensor_scalar(
                            hg[:, k, :], ob[:, k, :, ot], g_sb[:, k, t : t + 1],
                            None, op0=ALU.mult,
                        )
                    hcur = (lambda hg_: lambda k: hg_[:, k, :])(hg)
                    hfull = hg[:, :, :]
                else:
                    hcur = (lambda ob_, ot_: lambda k: ob_[:, k, :, ot_])(ob, ot)
                    hfull = ob[:, :, :, ot]
                if ot == OCH - 1:
                    sl = slice(t - OCH + 1, t + 1)
                    for k in range(2):
                        nc.sync.dma_start(out=out_d[k, :, :, sl], in_=ob[:, k, :, :])

    _fix_waits(nc)
    return nc


_BUILT = {}


def _get(T):
    if T not in _BUILT:
        _BUILT[T] = _build(T)
    return _BUILT[T]


def kernel(x, wm_key, weight_ih, weight_hh, bias_ih, bias_hh):
    x = np.asarray(x, np.float32)
    Bx, T, Ix = x.shape
    nc = _get(T)
    wih = np.ascontiguousarray(weight_ih.T.reshape(2, 128, M3), np.float32)
    whh = np.ascontiguousarray(weight_hh.T.reshape(2, 128, M3), np.float32)
    brow = (
        np.asarray(bias_ih, np.float32)
        + np.concatenate([np.asarray(bias_hh[: 2 * H], np.float32), np.zeros(H, np.float32)])
    ).reshape(1, M3)
    bhn = np.ascontiguousarray(
        np.tile(np.asarray(bias_hh[2 * H :], np.float32).reshape(2, 128, 1), (1, 1, BC))
    )
    wmk = np.ascontiguousarray(
        wm_key.transpose(2, 1, 0).reshape(2, 128, KL * KB), np.float32
    )
    in_maps = []
    for cidx in range(NCORE):
        xc = np.ascontiguousarray(
            x[cidx * BC : (cidx + 1) * BC].transpose(2, 1, 0).reshape(2, 128, T, BC)
        )
        in_maps.append(
            {"x": xc, "wih": wih, "whh": whh, "brow": brow, "bhn": bhn, "wmk": wmk}
        )
    res = run_bass_kernel_spmd(nc, in_maps, list(range(NCORE)))
    couts = np.stack([r["out"] for r in res.results], 0)  # [NC, 2, 128, BC, T]
    # out[t, c*BC+b, k*128+p] = couts[c, k, p, b, t]
    return np.ascontiguousarray(
        couts.transpose(4, 0, 3, 1, 2).reshape(T, B, H)
    )
